# revision 10
# baseline (speedup 1.0000x reference)
"""Trainium2 Bass kernel for nn_CRNNModel (GRU language-model-style CRNN).

Math (see reference):
  onehot = one_hot(inputs, 2); shifted = roll(onehot, 1, axis=time) with t=0 zeroed
  GRU (flax GRUCell) over N=256 steps, H=256, on B=1024 samples
  x = hs @ Wd + bd  (D=2)
  out[b] = 0.5 * sum_t log_softmax(x)[y] + 1j * sum_t pi*softsign(x @ Wp + bp)[y]

Key reductions used here:
  * D=2 -> the input matmul of the GRU is a rank-2 selection; it is folded into
    the PSUM accumulation via a single K=12 block-diagonal matmul per gate
    group (also folding the hidden bias b).
  * The readout only needs two scalars per (b, t):
        u = hs . (Wd[:,1]-Wd[:,0])   and   v = hs . (Wd[:,0]+Wd[:,1])
    log_softmax term  = -softplus((1-2y) * (u + bdelta))
    softsign argument = alpha_y*(v+bsigma) + beta_y*(u+bdelta) + bp_y
    Both are cheap elementwise expressions in (b, t) computed in a short
    epilogue phase.

Sharding: data parallel over the batch. 8 cores x 128 samples, identical
program, weights replicated; no collectives. Inputs are sharded / outputs
gathered on the host.
"""

import os
import sys

import numpy as np

sys.path.insert(0, "/opt/trn_rl_repo")

import concourse.bass as bass  # noqa: E402
import concourse.tile as tile  # noqa: E402
from concourse import bacc, mybir  # noqa: E402
from concourse import bass_utils  # noqa: E402
from concourse.masks import make_identity  # noqa: E402

F32 = mybir.dt.float32
AF = mybir.ActivationFunctionType
ALU = mybir.AluOpType

B, N, H, D = 1024, 256, 256, 2
NCORES = 8
BC = B // NCORES  # 128 samples per core
G = 3 * H  # 768 gate rows

# module-level stash so a test harness can inspect profiling info
LAST_RESULTS = None
_PROGRAM_CACHE = {}


def _scalars(Wd, bd, Wp, bp):
    """Host-side scalar constants for the epilogue."""
    bdelta = float(bd[1] - bd[0])
    bsigma = float(bd[0] + bd[1])
    # q_y = alpha_y * S + beta_y * delta + bp_y   with S=x0+x1, delta=x1-x0
    a0 = float((Wp[0, 0] + Wp[1, 0]) * 0.5)
    a1 = float((Wp[0, 1] + Wp[1, 1]) * 0.5)
    b0 = float((Wp[1, 0] - Wp[0, 0]) * 0.5)
    b1 = float((Wp[1, 1] - Wp[0, 1]) * 0.5)
    return dict(
        bdelta=bdelta,
        bsigma=bsigma,
        alpha0=a0,
        dalpha=a1 - a0,
        beta0=b0,
        dbeta=b1 - b0,
        bp0=float(bp[0]),
        dbp=float(bp[1] - bp[0]),
    )


def _build_program(n_steps, sc):
    """Build the per-core Bass/Tile program (identical on all cores)."""
    nc = bacc.Bacc(
        "TRN2", target_bir_lowering=False, debug=False, num_devices=NCORES
    )

    # ---- external I/O ----
    wh = nc.dram_tensor("wh", [H, G], F32, kind="ExternalInput").ap()
    augw_rz = nc.dram_tensor("augw_rz", [12, 128], F32, kind="ExternalInput").ap()
    augw_n = nc.dram_tensor("augw_n", [12, 128], F32, kind="ExternalInput").ap()
    w2 = nc.dram_tensor("w2", [128, 4], F32, kind="ExternalInput").ap()
    aug = nc.dram_tensor("aug", [n_steps, 12, 512], F32, kind="ExternalInput").ap()
    m_in = nc.dram_tensor("m", [BC, n_steps], F32, kind="ExternalInput").ap()
    out = nc.dram_tensor("out", [BC, 2], F32, kind="ExternalOutput").ap()

    from contextlib import ExitStack

    with tile.TileContext(nc) as tc, ExitStack() as ctx:
        consts = ctx.enter_context(tc.tile_pool(name="consts", bufs=1))
        dram = ctx.enter_context(tc.tile_pool(name="dram", bufs=1, space="DRAM"))

        # persistent weights in SBUF
        wh_sb = consts.tile([128, 2 * G], F32)  # [k*768 + gatecol]
        nc.sync.dma_start(wh_sb[:, 0:G], wh[0:128, :])
        nc.sync.dma_start(wh_sb[:, G : 2 * G], wh[128:256, :])
        awrz_sb = consts.tile([12, 128], F32)
        nc.sync.dma_start(awrz_sb, augw_rz)
        awn_sb = consts.tile([12, 128], F32)
        nc.sync.dma_start(awn_sb, augw_n)
        w2_sb = consts.tile([128, 4], F32)
        nc.sync.dma_start(w2_sb, w2)
        ident = consts.tile([128, 128], F32)
        make_identity(nc, ident)

        uv_dram = dram.tile([n_steps, 2, BC], F32)

        # ---------------- recurrence ----------------
        loop_ctx = ExitStack()
        hpool = loop_ctx.enter_context(tc.tile_pool(name="h", bufs=3))
        augp = loop_ctx.enter_context(tc.tile_pool(name="augp", bufs=8))
        psg = loop_ctx.enter_context(tc.tile_pool(name="psg", bufs=2, space="PSUM"))
        psuv = loop_ctx.enter_context(tc.tile_pool(name="psuv", bufs=2, space="PSUM"))
        gp = loop_ctx.enter_context(tc.tile_pool(name="gates", bufs=2))
        uvst = loop_ctx.enter_context(tc.tile_pool(name="uvst", bufs=4))

        h = hpool.tile([128, 2 * BC], F32, tag="h")
        nc.vector.memset(h, 0.0)

        for t in range(n_steps):
            aug_t = augp.tile([12, 512], F32, tag="aug")
            nc.sync.dma_start(aug_t, aug[t])

            ps_rz = psg.tile([128, 512], F32, tag="rz")
            nc.tensor.matmul(ps_rz, awrz_sb, aug_t, start=True, stop=False)
            ps_n = psg.tile([128, 512], F32, tag="n")
            nc.tensor.matmul(ps_n, awn_sb, aug_t, start=True, stop=False)

            # Middle matmuls accumulate element-wise into disjoint column
            # blocks; only one "stop carrier" per bank keeps the simulator's
            # bank-granular group bookkeeping consistent.
            for mchunk in range(6):
                if mchunk < 4:
                    dest = ps_rz[:, mchunk * 128 : (mchunk + 1) * 128]
                else:
                    dest = ps_n[:, (mchunk - 4) * 128 : (mchunk - 3) * 128]
                for k in range(2):
                    carrier = (mchunk in (3, 5)) and k == 1
                    nc.tensor.matmul(
                        dest,
                        wh_sb[:, k * G + mchunk * 128 : k * G + (mchunk + 1) * 128],
                        h[:, k * BC : (k + 1) * BC],
                        start=False,
                        stop=carrier,
                        skip_group_check=not carrier,
                    )

            rz = gp.tile([128, 512], F32, tag="rz_s")
            nc.scalar.activation(rz, ps_rz, AF.Sigmoid)
            u = gp.tile([128, 256], F32, tag="u")
            nc.vector.tensor_mul(u, rz[:, 0:256], ps_n[:, 0:256])
            w_ = gp.tile([128, 256], F32, tag="w")
            nc.vector.tensor_add(w_, u, ps_n[:, 256:512])
            nt = gp.tile([128, 256], F32, tag="nt")
            nc.scalar.activation(nt, w_, AF.Tanh)
            dd = gp.tile([128, 256], F32, tag="dd")
            nc.vector.tensor_sub(dd, h, nt)
            ee = gp.tile([128, 256], F32, tag="ee")
            nc.vector.tensor_mul(ee, rz[:, 256:512], dd)
            h2 = hpool.tile([128, 2 * BC], F32, tag="h")
            nc.vector.tensor_add(h2, nt, ee)

            ps_uv = psuv.tile([2, BC], F32, tag="uv")
            nc.tensor.matmul(ps_uv, w2_sb[:, 0:2], h2[:, 0:BC], start=True, stop=False)
            nc.tensor.matmul(
                ps_uv, w2_sb[:, 2:4], h2[:, BC : 2 * BC], start=False, stop=True
            )
            uvt = uvst.tile([2, BC], F32, tag="uvt")
            nc.scalar.copy(uvt, ps_uv)
            nc.sync.dma_start(uv_dram[t], uvt)

            h = h2

        loop_ctx.close()

        # ---------------- epilogue ----------------
        p3 = ctx.enter_context(tc.tile_pool(name="p3", bufs=1))
        p3t = ctx.enter_context(tc.tile_pool(name="p3t", bufs=2))
        psp3 = ctx.enter_context(tc.tile_pool(name="psp3", bufs=2, space="PSUM"))

        ntc = n_steps // 128 if n_steps >= 128 else 1
        tcw = min(n_steps, 128)
        U = p3.tile([128, n_steps], F32)
        V = p3.tile([128, n_steps], F32)
        for half, dst in ((0, U), (1, V)):
            for j in range(ntc):
                tmp = p3t.tile([128, BC], F32, tag="tr_in")
                nc.sync.dma_start(
                    tmp[0:tcw, :], uv_dram[j * tcw : (j + 1) * tcw, half, :]
                )
                pst = psp3.tile([128, 128], F32, tag="tr")
                nc.tensor.transpose(pst[:, 0:tcw], tmp[0:tcw, :], ident[0:tcw, 0:tcw])
                nc.vector.tensor_copy(dst[:, j * tcw : (j + 1) * tcw], pst[:, 0:tcw])

        mt = p3.tile([128, n_steps], F32)
        nc.sync.dma_start(mt[0:BC, :], m_in)

        a = p3.tile([128, n_steps], F32)
        nc.vector.tensor_scalar_add(a, U, sc["bdelta"])
        s = p3.tile([128, n_steps], F32)
        nc.vector.tensor_scalar(s, mt, -2.0, 1.0, ALU.mult, ALU.add)
        sa = p3.tile([128, n_steps], F32)
        nc.vector.tensor_mul(sa, s, a)
        sl = p3.tile([128, 1], F32)
        ex = p3.tile([128, n_steps], F32)
        nc.scalar.activation(ex, sa, AF.Exp)
        lt = p3.tile([128, n_steps], F32)
        nc.scalar.activation(lt, ex, AF.Ln, bias=1.0, accum_out=sl)

        vp = p3.tile([128, n_steps], F32)
        nc.vector.tensor_scalar_add(vp, V, sc["bsigma"])
        t1 = p3.tile([128, n_steps], F32)
        nc.vector.tensor_scalar(t1, mt, sc["dalpha"], sc["alpha0"], ALU.mult, ALU.add)
        t2 = p3.tile([128, n_steps], F32)
        nc.vector.tensor_mul(t2, t1, vp)
        t3 = p3.tile([128, n_steps], F32)
        nc.vector.tensor_scalar(t3, mt, sc["dbeta"], sc["beta0"], ALU.mult, ALU.add)
        t4 = p3.tile([128, n_steps], F32)
        nc.vector.tensor_mul(t4, t3, a)
        q = p3.tile([128, n_steps], F32)
        nc.vector.tensor_add(q, t2, t4)
        t5 = p3.tile([128, n_steps], F32)
        nc.vector.tensor_scalar(t5, mt, sc["dbp"], sc["bp0"], ALU.mult, ALU.add)
        q2 = p3.tile([128, n_steps], F32)
        nc.vector.tensor_add(q2, q, t5)

        aq = p3.tile([128, n_steps], F32)
        nc.scalar.activation(aq, q2, AF.Abs)
        dq = p3.tile([128, n_steps], F32)
        nc.vector.tensor_scalar_add(dq, aq, 1.0)
        rq = p3.tile([128, n_steps], F32)
        nc.vector.reciprocal(rq, dq)
        sp = p3.tile([128, 1], F32)
        ph = p3.tile([128, n_steps], F32)
        nc.vector.scalar_tensor_tensor(
            ph, q2, 1.0, rq, ALU.mult, ALU.mult, accum_out=sp
        )

        o = p3.tile([128, 2], F32)
        nc.vector.tensor_scalar_mul(o[:, 0:1], sl, -0.5)
        nc.vector.tensor_scalar_mul(o[:, 1:2], sp, float(np.pi))
        nc.sync.dma_start(out, o[0:BC, :])

    nc.compile()
    names = dict(
        inputs=["wh", "augw_rz", "augw_n", "w2", "aug", "m"], output="out"
    )
    return nc, names


def _host_prep(inputs, Wi, Wh, b, Wd, bd, Wp, bp, n_steps, n_cores):
    """Build shared weight tensors + per-core input maps (numpy)."""
    y = np.asarray(inputs)
    bc = y.shape[0] // n_cores

    Wi = np.asarray(Wi, np.float32)
    Wh = np.asarray(Wh, np.float32)
    b = np.asarray(b, np.float32)
    Wd = np.asarray(Wd, np.float32)

    wh = np.ascontiguousarray(Wh)

    augw_rz = np.zeros((12, 128), np.float32)
    for j in range(4):
        cols = slice(j * 128, (j + 1) * 128)
        augw_rz[3 * j + 0] = Wi[0, cols]
        augw_rz[3 * j + 1] = Wi[1, cols]
        augw_rz[3 * j + 2] = b[cols]

    augw_n = np.zeros((12, 128), np.float32)
    for j in range(2):  # hn bias blocks
        cols = slice(512 + j * 128, 512 + (j + 1) * 128)
        augw_n[3 * j + 2] = b[cols]
    for j in range(2, 4):  # inn blocks
        cols = slice(512 + (j - 2) * 128, 512 + (j - 1) * 128)
        augw_n[3 * j + 0] = Wi[0, cols]
        augw_n[3 * j + 1] = Wi[1, cols]

    wdelta = Wd[:, 1] - Wd[:, 0]
    wsigma = Wd[:, 0] + Wd[:, 1]
    w2 = np.zeros((128, 4), np.float32)
    w2[:, 0] = wdelta[0:128]
    w2[:, 1] = wsigma[0:128]
    w2[:, 2] = wdelta[128:256]
    w2[:, 3] = wsigma[128:256]

    shared = dict(wh=wh, augw_rz=augw_rz, augw_n=augw_n, w2=w2)

    in_maps = []
    for c in range(n_cores):
        yc = y[c * bc : (c + 1) * bc]  # [bc, n_steps]
        aug = np.zeros((n_steps, 12, 512), np.float32)
        for j in range(4):
            cols = slice(j * 128, j * 128 + bc)
            # shifted teacher-forced input: step t sees onehot(y[:, t-1]); t=0 is zeros
            aug[1:, 3 * j + 0, cols] = (yc[:, : n_steps - 1] == 0).T
            aug[1:, 3 * j + 1, cols] = (yc[:, : n_steps - 1] == 1).T
            aug[:, 3 * j + 2, cols] = 1.0
        m = np.ascontiguousarray(yc.astype(np.float32))
        in_maps.append(dict(shared, aug=aug, m=m))
    return in_maps


def kernel(inputs, Wi, Wh, b, Wd, bd, Wp, bp):
    global LAST_RESULTS
    n_steps = np.asarray(inputs).shape[1]
    sc = _scalars(np.asarray(Wd, np.float32), np.asarray(bd, np.float32),
                  np.asarray(Wp, np.float32), np.asarray(bp, np.float32))

    key = (n_steps, tuple(sorted(sc.items())))
    if key not in _PROGRAM_CACHE:
        _PROGRAM_CACHE.clear()
        _PROGRAM_CACHE[key] = _build_program(n_steps, sc)
    nc, names = _PROGRAM_CACHE[key]

    in_maps = _host_prep(inputs, Wi, Wh, b, Wd, bd, Wp, bp, n_steps, NCORES)
    trace = bool(int(os.environ.get("KERNEL_TRACE", "0")))
    res = bass_utils.run_bass_kernel_spmd(
        nc, in_maps, core_ids=list(range(NCORES)), trace=trace
    )
    LAST_RESULTS = res

    outs = [r["out"] for r in res.results]
    full = np.concatenate(outs, axis=0)  # [B, 2]
    return (full[:, 0] + 1j * full[:, 1]).astype(np.complex64)
